# revision 10
# baseline (speedup 1.0000x reference)
"""BiDAF attention kernel for Trainium2 (8 NeuronCores, data-parallel over batch).

Problem (per full input): B=16, L=M=1024, H=128
  s  = text@tw + (mod@mw).T + (text*tmw)@mod.T + bias          (B, L, M)
  p1 = softmax_M(mmask*s + (1-mmask)*NEG)
  p2 = softmax_L(tmask*s + (1-tmask)*NEG)
  a  = p1 @ mod
  b  = p1 @ p2.T @ text        (computed as p1 @ (p2.T @ text))
  out = [text, a, text*a, text*b]                               (B, L, 4H)

Key algebraic facts used:
  * softmax_M is invariant to per-row (per-l) shifts: s0 & bias drop from p1.
  * softmax_L is invariant to per-column (per-m) shifts: s1 & bias drop from p2.
  * masking with {0,1} is equivalent to adding (mask-1)*30000 before exp.
  * appending a ones-column to the rhs of the p1/p2 contraction matmuls
    yields the softmax denominators for free (an extra output column).

Each of the 8 cores processes 2 batch items; no cross-core communication.
"""

import numpy as np

B, L, M, H = 16, 1024, 1024, 128
NCORES = 8
BPC = B // NCORES  # batches per core
P = 128
LT, MT = L // P, M // P
NEGB = 30000.0

_CACHE = {}


def _build():
    """Builds the per-core Bass program (SPMD: same NEFF on all 8 cores)."""
    from contextlib import ExitStack

    import concourse.bass as bass
    import concourse.mybir as mybir
    import concourse.tile as tile
    from concourse import bacc
    from concourse.bass import ts
    from concourse.masks import make_identity

    f32 = mybir.dt.float32
    i32 = mybir.dt.int32
    Exp = mybir.ActivationFunctionType.Exp
    Alu = mybir.AluOpType

    nc = bacc.Bacc(name="bidaf8")
    text = nc.dram_tensor("text", (BPC, L, H), f32, kind="ExternalInput").ap()
    modl = nc.dram_tensor("modality", (BPC, M, H), f32, kind="ExternalInput").ap()
    tmask = nc.dram_tensor("text_mask", (BPC, L), i32, kind="ExternalInput").ap()
    mmask = nc.dram_tensor("modality_mask", (BPC, M), i32, kind="ExternalInput").ap()
    wt = nc.dram_tensor("w_text", (H, 1), f32, kind="ExternalInput").ap()
    wm = nc.dram_tensor("w_mod", (H, 1), f32, kind="ExternalInput").ap()
    wtm = nc.dram_tensor("w_tm", (H, 1), f32, kind="ExternalInput").ap()
    out = nc.dram_tensor("out", (BPC, L, 4 * H), f32, kind="ExternalOutput").ap()

    def rep_rows(col_ap):
        # (H, 1) DRAM column -> broadcast AP read as (P, H): every partition
        # reads the same H contiguous floats.
        return bass.AP(tensor=col_ap.tensor, offset=col_ap.offset,
                       ap=[[0, P], col_ap.ap[0]])

    with tile.TileContext(nc) as tc, ExitStack() as ctx:
        const = ctx.enter_context(tc.tile_pool(name="const", bufs=1))
        oper = ctx.enter_context(tc.tile_pool(name="oper", bufs=2))
        big = ctx.enter_context(tc.tile_pool(name="big", bufs=1))
        small = ctx.enter_context(tc.tile_pool(name="small", bufs=2))
        outp = ctx.enter_context(tc.tile_pool(name="outp", bufs=4))
        ps_s = ctx.enter_context(tc.tile_pool(name="ps_s", bufs=2, space="PSUM"))
        ps_q = ctx.enter_context(tc.tile_pool(name="ps_q", bufs=4, space="PSUM"))

        ident = const.tile([P, P], f32)
        make_identity(nc, ident)
        wtm_sb = const.tile([P, 1], f32)
        nc.sync.dma_start(wtm_sb, wtm)
        # replicated weight rows for DVE row-dot-products
        wt_rep = const.tile([P, H], f32)
        nc.gpsimd.dma_start(wt_rep, rep_rows(wt))
        wm_rep = const.tile([P, H], f32)
        nc.gpsimd.dma_start(wm_rep, rep_rows(wm))

        for b in range(BPC):
            # ---- natural-layout loads, with a ones column appended ----
            txt = oper.tile([P, LT, H + 1], f32, tag="txt")
            nc.vector.memset(txt[:, :, H : H + 1], 1.0)
            nc.sync.dma_start(txt[:, :, :H], text[b].rearrange("(o p) h -> p o h", p=P))
            # mod rows followed by ones column and (later) wq columns:
            # rhs of the fused a/b matmul is [mod | 1 | wq]  (257 wide)
            modwq = big.tile([P, MT, 2 * H + 1], f32, tag="modwq")
            nc.vector.memset(modwq[:, :, H : H + 1], 1.0)
            nc.sync.dma_start(modwq[:, :, :H],
                              modl[b].rearrange("(o p) h -> p o h", p=P))

            # ---- masks: load natural (chunk, 128), convert, PE-transpose ----
            tmki = small.tile([LT, P], i32, tag="tmki")
            nc.sync.dma_start(tmki, tmask[b].rearrange("(o p) -> o p", p=P))
            tmkf = small.tile([LT, P], f32, tag="tmkf")
            nc.vector.tensor_copy(tmkf, tmki)
            mmki = small.tile([MT, P], i32, tag="mmki")
            nc.sync.dma_start(mmki, mmask[b].rearrange("(o p) -> o p", p=P))
            mmkf = small.tile([MT, P], f32, tag="mmkf")
            nc.vector.tensor_copy(mmkf, mmki)

            bias2 = small.tile([P, LT], f32, tag="bias2")  # per-l: s0+(tm-1)*NEGB
            tp = ps_q.tile([P, P], f32, tag="q")
            nc.tensor.transpose(tp[:, :LT], tmkf, ident[:LT, :LT])
            nc.vector.tensor_scalar(bias2, tp[:, :LT], 1.0, NEGB,
                                    op0=Alu.subtract, op1=Alu.mult)
            bias1 = small.tile([P, MT], f32, tag="bias1")  # per-m: s1+(mm-1)*NEGB
            tp = ps_q.tile([P, P], f32, tag="q")
            nc.tensor.transpose(tp[:, :MT], mmkf, ident[:MT, :MT])
            nc.vector.tensor_scalar(bias1, tp[:, :MT], 1.0, NEGB,
                                    op0=Alu.subtract, op1=Alu.mult)

            # ---- s0/s1 row-dots on DVE, then add into the bias columns ----
            # s0[l] = sum_h txt[l, h] * wt[h]
            s0col = small.tile([P, LT], f32, tag="s0col")
            for j in range(LT):
                scr = small.tile([P, H], f32, tag="scr")
                nc.vector.scalar_tensor_tensor(
                    out=scr, in0=txt[:, j, :H], scalar=1.0, in1=wt_rep,
                    op0=Alu.mult, op1=Alu.mult,
                    accum_out=s0col[:, j : j + 1])
            nc.vector.tensor_add(bias2, bias2, s0col)
            s1col = small.tile([P, MT], f32, tag="s1col")
            for k in range(MT):
                scr = small.tile([P, H], f32, tag="scr")
                nc.vector.scalar_tensor_tensor(
                    out=scr, in0=modwq[:, k, :H], scalar=1.0, in1=wm_rep,
                    op0=Alu.mult, op1=Alu.mult,
                    accum_out=s1col[:, k : k + 1])
            nc.vector.tensor_add(bias1, bias1, s1col)

            # ---- transposes: txtT/modT are (H, L)/(H, M) ----
            txtT = oper.tile([P, L], f32, tag="txtT")
            modT = oper.tile([P, M], f32, tag="modT")
            for j in range(LT):
                tp = ps_q.tile([P, P], f32, tag="q")
                nc.tensor.transpose(tp, txt[:, j, :H], ident)
                nc.vector.tensor_copy(txtT[:, ts(j, P)], tp)
            for k in range(MT):
                tp = ps_q.tile([P, P], f32, tag="q")
                nc.tensor.transpose(tp, modwq[:, k, :H], ident)
                nc.vector.tensor_copy(modT[:, ts(k, P)], tp)

            # ---- X^T = txtT * w_tm (per-partition h scale), in place ----
            nc.vector.tensor_scalar_mul(txtT, txtT, wtm_sb)

            # ---- E1T[m, l] = exp(s2T + bias1[m])  (M on partitions) ----
            E1T = big.tile([P, MT, L], f32, tag="E1T")
            for k in range(MT):
                sp = ps_s.tile([P, L], f32, tag="s")
                nc.tensor.matmul(sp[:, 0:512], modT[:, ts(k, P)], txtT[:, 0:512],
                                 start=True, stop=True)
                nc.tensor.matmul(sp[:, 512:1024], modT[:, ts(k, P)], txtT[:, 512:1024],
                                 start=True, stop=True)
                nc.scalar.activation(E1T[:, k, :], sp, Exp,
                                     bias=bias1[:, k : k + 1], scale=1.0)

            # ---- E2[l, m] = exp(s2 + bias2[l])  (L on partitions) ----
            E2 = big.tile([P, LT, M], f32, tag="E2")
            for j in range(LT):
                sp = ps_s.tile([P, M], f32, tag="s")
                nc.tensor.matmul(sp[:, 0:512], txtT[:, ts(j, P)], modT[:, 0:512],
                                 start=True, stop=True)
                nc.tensor.matmul(sp[:, 512:1024], txtT[:, ts(j, P)], modT[:, 512:1024],
                                 start=True, stop=True)
                nc.scalar.activation(E2[:, j, :], sp, Exp,
                                     bias=bias2[:, j : j + 1], scale=1.0)

            # ---- wq = p2.T @ text: q2[m,:] = E2.T @ [text|1]; wq = q2/D2 ----
            for k in range(MT):
                qp = ps_q.tile([P, H + 1], f32, tag="q")
                for j in range(LT):
                    nc.tensor.matmul(qp, E2[:, j, ts(k, P)], txt[:, j, :],
                                     start=(j == 0), stop=(j == LT - 1))
                rec = small.tile([P, 1], f32, tag="rec2")
                nc.vector.reciprocal(rec, qp[:, H : H + 1])
                nc.vector.tensor_scalar_mul(modwq[:, k, H + 1 :], qp[:, :H], rec)

            # ---- fused [a | D1 | b] = E1 @ [mod | 1 | wq]; assemble out ----
            for j in range(LT):
                pa = ps_q.tile([P, 2 * H + 1], f32, tag="q")
                for k in range(MT):
                    nc.tensor.matmul(pa, E1T[:, k, ts(j, P)], modwq[:, k, :],
                                     start=(k == 0), stop=(k == MT - 1))
                rec1 = small.tile([P, 1], f32, tag="rec1")
                nc.vector.reciprocal(rec1, pa[:, H : H + 1])
                o = outp.tile([P, 4 * H], f32, tag="o")
                nc.gpsimd.tensor_copy(o[:, 0:H], txt[:, j, :H])
                nc.vector.tensor_scalar_mul(o[:, H : 2 * H], pa[:, :H], rec1)
                nc.vector.tensor_mul(o[:, 2 * H : 3 * H], o[:, H : 2 * H],
                                     txt[:, j, :H])
                nc.vector.tensor_scalar_mul(o[:, 3 * H : 4 * H], pa[:, H + 1 :], rec1)
                nc.vector.tensor_mul(o[:, 3 * H : 4 * H], o[:, 3 * H : 4 * H],
                                     txt[:, j, :H])
                nc.sync.dma_start(
                    out[b].rearrange("(o p) c -> p o c", p=P)[:, j, :], o
                )
    nc.compile()
    return nc


def get_nc():
    if "nc" not in _CACHE:
        _CACHE["nc"] = _build()
    return _CACHE["nc"]


def make_in_maps(text, modality, text_mask, modality_mask,
                 text_weight, modality_weight, text_modality_weight):
    text = np.ascontiguousarray(np.asarray(text, dtype=np.float32))
    modality = np.ascontiguousarray(np.asarray(modality, dtype=np.float32))
    text_mask = np.ascontiguousarray(np.asarray(text_mask, dtype=np.int32))
    modality_mask = np.ascontiguousarray(np.asarray(modality_mask, dtype=np.int32))
    wt = np.ascontiguousarray(np.asarray(text_weight, dtype=np.float32).reshape(H, 1))
    wm = np.ascontiguousarray(
        np.asarray(modality_weight, dtype=np.float32).reshape(H, 1))
    wtm = np.ascontiguousarray(
        np.asarray(text_modality_weight, dtype=np.float32).reshape(H, 1))
    in_maps = []
    for c in range(NCORES):
        sl = slice(BPC * c, BPC * (c + 1))
        in_maps.append({
            "text": np.ascontiguousarray(text[sl]),
            "modality": np.ascontiguousarray(modality[sl]),
            "text_mask": np.ascontiguousarray(text_mask[sl]),
            "modality_mask": np.ascontiguousarray(modality_mask[sl]),
            "w_text": wt,
            "w_mod": wm,
            "w_tm": wtm,
        })
    return in_maps


def kernel(text, modality, text_mask, modality_mask,
           text_weight, modality_weight, text_modality_weight, bias,
           trace=False):
    from concourse.bass_utils import run_bass_kernel_spmd

    nc = get_nc()
    in_maps = make_in_maps(text, modality, text_mask, modality_mask,
                           text_weight, modality_weight, text_modality_weight)
    res = run_bass_kernel_spmd(nc, in_maps, core_ids=list(range(NCORES)),
                               trace=trace)
    outp = np.concatenate([r["out"] for r in res.results], axis=0)
    if trace:
        kernel.last_result = res
    return outp


# revision 12
# speedup vs baseline: 1.8954x; 1.8954x over previous
"""BiDAF attention kernel for Trainium2 (8 NeuronCores, data-parallel over batch).

Problem (per full input): B=16, L=M=1024, H=128
  s  = text@tw + (mod@mw).T + (text*tmw)@mod.T + bias          (B, L, M)
  p1 = softmax_M(mmask*s + (1-mmask)*NEG)
  p2 = softmax_L(tmask*s + (1-tmask)*NEG)
  a  = p1 @ mod
  b  = p1 @ p2.T @ text        (computed as p1 @ (p2.T @ text))
  out = [text, a, text*a, text*b]                               (B, L, 4H)

Key facts used:
  * softmax_M is invariant to per-row (per-l) shifts: s0 & bias drop from p1.
  * softmax_L is invariant to per-column (per-m) shifts: s1 & bias drop from p2.
  * masking with {0,1} is equivalent to adding (mask-1)*30000 before exp.
  * appending a ones-column to the rhs of the p1/p2 contraction matmuls
    yields the softmax denominators for free (an extra output column).
  * fp32 matmuls run 2-pass (LOW_HIGH) on trn2 — all matmul operands are
    kept in bf16 (PSUM accumulation and softmax normalization stay fp32).

Each of the 8 cores processes 2 batch items; no cross-core communication.
"""

import numpy as np

B, L, M, H = 16, 1024, 1024, 128
NCORES = 8
BPC = B // NCORES  # batches per core
P = 128
LT, MT = L // P, M // P
NEGB = 30000.0

_CACHE = {}


def _build():
    """Builds the per-core Bass program (SPMD: same NEFF on all 8 cores)."""
    from contextlib import ExitStack

    import concourse.bass as bass
    import concourse.mybir as mybir
    import concourse.tile as tile
    from concourse import bacc
    from concourse.bass import ts
    from concourse.masks import make_identity

    f32 = mybir.dt.float32
    bf16 = mybir.dt.bfloat16
    i32 = mybir.dt.int32
    Exp = mybir.ActivationFunctionType.Exp
    Alu = mybir.AluOpType

    nc = bacc.Bacc(name="bidaf8")
    text = nc.dram_tensor("text", (BPC, L, H), f32, kind="ExternalInput").ap()
    modl = nc.dram_tensor("modality", (BPC, M, H), f32, kind="ExternalInput").ap()
    tmask = nc.dram_tensor("text_mask", (BPC, L), i32, kind="ExternalInput").ap()
    mmask = nc.dram_tensor("modality_mask", (BPC, M), i32, kind="ExternalInput").ap()
    wt = nc.dram_tensor("w_text", (H, 1), f32, kind="ExternalInput").ap()
    wm = nc.dram_tensor("w_mod", (H, 1), f32, kind="ExternalInput").ap()
    wtm = nc.dram_tensor("w_tm", (H, 1), f32, kind="ExternalInput").ap()
    out = nc.dram_tensor("out", (BPC, L, 4 * H), f32, kind="ExternalOutput").ap()

    def rep_rows(col_ap):
        # (H, 1) DRAM column -> broadcast AP read as (P, H): every partition
        # reads the same H contiguous floats. (gpsimd DMA only)
        return bass.AP(tensor=col_ap.tensor, offset=col_ap.offset,
                       ap=[[0, P], col_ap.ap[0]])

    with tile.TileContext(nc) as tc, ExitStack() as ctx:
        const = ctx.enter_context(tc.tile_pool(name="const", bufs=1))
        oper = ctx.enter_context(tc.tile_pool(name="oper", bufs=2))
        big = ctx.enter_context(tc.tile_pool(name="big", bufs=2))
        small = ctx.enter_context(tc.tile_pool(name="small", bufs=2))
        outp = ctx.enter_context(tc.tile_pool(name="outp", bufs=4))
        ps_s = ctx.enter_context(tc.tile_pool(name="ps_s", bufs=2, space="PSUM"))
        ps_q = ctx.enter_context(tc.tile_pool(name="ps_q", bufs=4, space="PSUM"))

        ident = const.tile([P, P], f32)
        make_identity(nc, ident)
        ident16 = const.tile([P, P], bf16)
        make_identity(nc, ident16)
        wtm_sb = const.tile([P, 1], f32)
        nc.sync.dma_start(wtm_sb, wtm)
        # replicated weight rows for DVE row-dot-products
        wt_rep = const.tile([P, H], f32)
        nc.gpsimd.dma_start(wt_rep, rep_rows(wt))
        wm_rep = const.tile([P, H], f32)
        nc.gpsimd.dma_start(wm_rep, rep_rows(wm))

        for b in range(BPC):
            # ---- natural-layout loads (fp32) + bf16 casts ----
            txt = oper.tile([P, LT, H], f32, tag="txt")
            nc.sync.dma_start(txt, text[b].rearrange("(o p) h -> p o h", p=P))
            mods = oper.tile([P, MT, H], f32, tag="mods")
            nc.sync.dma_start(mods, modl[b].rearrange("(o p) h -> p o h", p=P))

            # bf16 [text | 1]: rhs of the q2 matmul
            txt16 = oper.tile([P, LT, H + 1], bf16, tag="txt16")
            nc.vector.memset(txt16[:, :, H : H + 1], 1.0)
            nc.vector.tensor_copy(txt16[:, :, :H], txt)
            # bf16 [mod | 1 | wq]: rhs of the fused a/b matmul (257 wide)
            modwq = big.tile([P, MT, 2 * H + 1], bf16, tag="modwq")
            nc.vector.memset(modwq[:, :, H : H + 1], 1.0)
            nc.vector.tensor_copy(modwq[:, :, :H], mods)

            # ---- masks: load natural (chunk, 128), convert, PE-transpose ----
            tmki = small.tile([LT, P], i32, tag="tmki")
            nc.sync.dma_start(tmki, tmask[b].rearrange("(o p) -> o p", p=P))
            tmkf = small.tile([LT, P], f32, tag="tmkf")
            nc.vector.tensor_copy(tmkf, tmki)
            mmki = small.tile([MT, P], i32, tag="mmki")
            nc.sync.dma_start(mmki, mmask[b].rearrange("(o p) -> o p", p=P))
            mmkf = small.tile([MT, P], f32, tag="mmkf")
            nc.vector.tensor_copy(mmkf, mmki)

            bias2 = small.tile([P, LT], f32, tag="bias2")  # per-l: s0+(tm-1)*NEGB
            tp = ps_q.tile([P, P], f32, tag="q")
            nc.tensor.transpose(tp[:, :LT], tmkf, ident[:LT, :LT])
            nc.vector.tensor_scalar(bias2, tp[:, :LT], 1.0, NEGB,
                                    op0=Alu.subtract, op1=Alu.mult)
            bias1 = small.tile([P, MT], f32, tag="bias1")  # per-m: s1+(mm-1)*NEGB
            tp = ps_q.tile([P, P], f32, tag="q")
            nc.tensor.transpose(tp[:, :MT], mmkf, ident[:MT, :MT])
            nc.vector.tensor_scalar(bias1, tp[:, :MT], 1.0, NEGB,
                                    op0=Alu.subtract, op1=Alu.mult)

            # ---- s0/s1 row-dots on DVE (fp32), added into the bias columns ----
            s0col = small.tile([P, LT], f32, tag="s0col")
            for j in range(LT):
                scr = small.tile([P, H], f32, tag="scr")
                nc.vector.scalar_tensor_tensor(
                    out=scr, in0=txt[:, j, :], scalar=1.0, in1=wt_rep,
                    op0=Alu.mult, op1=Alu.mult,
                    accum_out=s0col[:, j : j + 1])
            nc.vector.tensor_add(bias2, bias2, s0col)
            s1col = small.tile([P, MT], f32, tag="s1col")
            for k in range(MT):
                scr = small.tile([P, H], f32, tag="scr")
                nc.vector.scalar_tensor_tensor(
                    out=scr, in0=mods[:, k, :], scalar=1.0, in1=wm_rep,
                    op0=Alu.mult, op1=Alu.mult,
                    accum_out=s1col[:, k : k + 1])
            nc.vector.tensor_add(bias1, bias1, s1col)

            # ---- transposes (bf16): txtT/modT are (H, L)/(H, M) ----
            txtT = oper.tile([P, L], bf16, tag="txtT")
            modT = oper.tile([P, M], bf16, tag="modT")
            for j in range(LT):
                tp = ps_q.tile([P, P], bf16, tag="q")
                nc.tensor.transpose(tp, txt16[:, j, :H], ident16)
                nc.vector.tensor_copy(txtT[:, ts(j, P)], tp)
            for k in range(MT):
                tp = ps_q.tile([P, P], bf16, tag="q")
                nc.tensor.transpose(tp, modwq[:, k, :H], ident16)
                nc.vector.tensor_copy(modT[:, ts(k, P)], tp)

            # ---- X^T = txtT * w_tm (per-partition h scale), in place ----
            nc.vector.tensor_scalar_mul(txtT, txtT, wtm_sb)

            # ---- E1T[m, l] = exp(s2T + bias1[m])  (M on partitions, bf16) ----
            E1T = big.tile([P, MT, L], bf16, tag="E1T")
            for k in range(MT):
                sp = ps_s.tile([P, L], f32, tag="s")
                nc.tensor.matmul(sp[:, 0:512], modT[:, ts(k, P)], txtT[:, 0:512],
                                 start=True, stop=True)
                nc.tensor.matmul(sp[:, 512:1024], modT[:, ts(k, P)], txtT[:, 512:1024],
                                 start=True, stop=True)
                nc.scalar.activation(E1T[:, k, :], sp, Exp,
                                     bias=bias1[:, k : k + 1], scale=1.0)

            # ---- E2[l, m] = exp(s2 + bias2[l])  (L on partitions, bf16) ----
            E2 = big.tile([P, LT, M], bf16, tag="E2")
            for j in range(LT):
                sp = ps_s.tile([P, M], f32, tag="s")
                nc.tensor.matmul(sp[:, 0:512], txtT[:, ts(j, P)], modT[:, 0:512],
                                 start=True, stop=True)
                nc.tensor.matmul(sp[:, 512:1024], txtT[:, ts(j, P)], modT[:, 512:1024],
                                 start=True, stop=True)
                nc.scalar.activation(E2[:, j, :], sp, Exp,
                                     bias=bias2[:, j : j + 1], scale=1.0)

            # ---- wq = p2.T @ text: q2[m,:] = E2.T @ [text|1]; wq = q2/D2 ----
            for k in range(MT):
                qp = ps_q.tile([P, H + 1], f32, tag="q")
                for j in range(LT):
                    nc.tensor.matmul(qp, E2[:, j, ts(k, P)], txt16[:, j, :],
                                     start=(j == 0), stop=(j == LT - 1))
                rec = small.tile([P, 1], f32, tag="rec2")
                nc.vector.reciprocal(rec, qp[:, H : H + 1])
                nc.vector.tensor_scalar_mul(modwq[:, k, H + 1 :], qp[:, :H], rec)

            # ---- fused [a | D1 | b] = E1 @ [mod | 1 | wq]; assemble out ----
            for j in range(LT):
                pa = ps_q.tile([P, 2 * H + 1], f32, tag="q")
                for k in range(MT):
                    nc.tensor.matmul(pa, E1T[:, k, ts(j, P)], modwq[:, k, :],
                                     start=(k == 0), stop=(k == MT - 1))
                rec1 = small.tile([P, 1], f32, tag="rec1")
                nc.vector.reciprocal(rec1, pa[:, H : H + 1])
                o = outp.tile([P, 4 * H], f32, tag="o")
                nc.gpsimd.tensor_copy(o[:, 0:H], txt[:, j, :])
                nc.vector.tensor_scalar_mul(o[:, H : 2 * H], pa[:, :H], rec1)
                nc.vector.tensor_mul(o[:, 2 * H : 3 * H], o[:, H : 2 * H],
                                     txt[:, j, :])
                nc.vector.tensor_scalar_mul(o[:, 3 * H : 4 * H], pa[:, H + 1 :], rec1)
                nc.vector.tensor_mul(o[:, 3 * H : 4 * H], o[:, 3 * H : 4 * H],
                                     txt[:, j, :])
                nc.sync.dma_start(
                    out[b].rearrange("(o p) c -> p o c", p=P)[:, j, :], o
                )
    nc.compile()
    return nc


def get_nc():
    if "nc" not in _CACHE:
        _CACHE["nc"] = _build()
    return _CACHE["nc"]


def make_in_maps(text, modality, text_mask, modality_mask,
                 text_weight, modality_weight, text_modality_weight):
    text = np.ascontiguousarray(np.asarray(text, dtype=np.float32))
    modality = np.ascontiguousarray(np.asarray(modality, dtype=np.float32))
    text_mask = np.ascontiguousarray(np.asarray(text_mask, dtype=np.int32))
    modality_mask = np.ascontiguousarray(np.asarray(modality_mask, dtype=np.int32))
    wt = np.ascontiguousarray(np.asarray(text_weight, dtype=np.float32).reshape(H, 1))
    wm = np.ascontiguousarray(
        np.asarray(modality_weight, dtype=np.float32).reshape(H, 1))
    wtm = np.ascontiguousarray(
        np.asarray(text_modality_weight, dtype=np.float32).reshape(H, 1))
    in_maps = []
    for c in range(NCORES):
        sl = slice(BPC * c, BPC * (c + 1))
        in_maps.append({
            "text": np.ascontiguousarray(text[sl]),
            "modality": np.ascontiguousarray(modality[sl]),
            "text_mask": np.ascontiguousarray(text_mask[sl]),
            "modality_mask": np.ascontiguousarray(modality_mask[sl]),
            "w_text": wt,
            "w_mod": wm,
            "w_tm": wtm,
        })
    return in_maps


def kernel(text, modality, text_mask, modality_mask,
           text_weight, modality_weight, text_modality_weight, bias,
           trace=False):
    from concourse.bass_utils import run_bass_kernel_spmd

    nc = get_nc()
    in_maps = make_in_maps(text, modality, text_mask, modality_mask,
                           text_weight, modality_weight, text_modality_weight)
    res = run_bass_kernel_spmd(nc, in_maps, core_ids=list(range(NCORES)),
                               trace=trace)
    outp = np.concatenate([r["out"] for r in res.results], axis=0)
    if trace:
        kernel.last_result = res
    return outp


# revision 13
# speedup vs baseline: 2.0581x; 1.0858x over previous
"""BiDAF attention kernel for Trainium2 (8 NeuronCores, data-parallel over batch).

Problem (per full input): B=16, L=M=1024, H=128
  s  = text@tw + (mod@mw).T + (text*tmw)@mod.T + bias          (B, L, M)
  p1 = softmax_M(mmask*s + (1-mmask)*NEG)
  p2 = softmax_L(tmask*s + (1-tmask)*NEG)
  a  = p1 @ mod
  b  = p1 @ p2.T @ text        (computed as p1 @ (p2.T @ text))
  out = [text, a, text*a, text*b]                               (B, L, 4H)

Key facts used:
  * softmax_M is invariant to per-row (per-l) shifts: s0 & bias drop from p1.
  * softmax_L is invariant to per-column (per-m) shifts: s1 & bias drop from p2.
  * masking with {0,1} is equivalent to adding (mask-1)*30000 before exp.
  * a ones-column appended to the rhs of the p1/p2 contraction matmuls
    yields the softmax denominators for free (an extra output column).
  * fp32 matmuls run 2-pass (LOW_HIGH) on trn2 — all matmul operands are
    kept in bf16 (PSUM accumulation and softmax normalization stay fp32).
  * rows are assigned to SBUF partitions as l = p*8+o ("(p o)" split), so
    every DRAM<->SBUF transfer is 4KB-contiguous per partition; all l/m
    contractions are permutation-invariant and the output is written back
    with the same mapping.

Each of the 8 cores processes 2 batch items; no cross-core communication.
"""

import numpy as np

B, L, M, H = 16, 1024, 1024, 128
NCORES = 8
BPC = B // NCORES  # batches per core
P = 128
LT, MT = L // P, M // P
NEGB = 30000.0

_CACHE = {}


def _build():
    """Builds the per-core Bass program (SPMD: same NEFF on all 8 cores)."""
    from contextlib import ExitStack

    import concourse.bass as bass
    import concourse.mybir as mybir
    import concourse.tile as tile
    from concourse import bacc
    from concourse.bass import ts
    from concourse.masks import make_identity

    f32 = mybir.dt.float32
    bf16 = mybir.dt.bfloat16
    i32 = mybir.dt.int32
    Exp = mybir.ActivationFunctionType.Exp
    Alu = mybir.AluOpType

    nc = bacc.Bacc(name="bidaf8")
    text = nc.dram_tensor("text", (BPC, L, H), f32, kind="ExternalInput").ap()
    modl = nc.dram_tensor("modality", (BPC, M, H), f32, kind="ExternalInput").ap()
    tmask = nc.dram_tensor("text_mask", (BPC, L), i32, kind="ExternalInput").ap()
    mmask = nc.dram_tensor("modality_mask", (BPC, M), i32, kind="ExternalInput").ap()
    wt = nc.dram_tensor("w_text", (H, 1), f32, kind="ExternalInput").ap()
    wm = nc.dram_tensor("w_mod", (H, 1), f32, kind="ExternalInput").ap()
    wtm = nc.dram_tensor("w_tm", (H, 1), f32, kind="ExternalInput").ap()
    out = nc.dram_tensor("out", (BPC, L, 4 * H), f32, kind="ExternalOutput").ap()

    def rep_rows(col_ap):
        # (H, 1) DRAM column -> broadcast AP read as (P, H): every partition
        # reads the same H contiguous floats. (gpsimd DMA only)
        return bass.AP(tensor=col_ap.tensor, offset=col_ap.offset,
                       ap=[[0, P], col_ap.ap[0]])

    with tile.TileContext(nc) as tc, ExitStack() as ctx:
        const = ctx.enter_context(tc.tile_pool(name="const", bufs=1))
        oper = ctx.enter_context(tc.tile_pool(name="oper", bufs=2))
        big = ctx.enter_context(tc.tile_pool(name="big", bufs=2))
        small = ctx.enter_context(tc.tile_pool(name="small", bufs=2))
        outp = ctx.enter_context(tc.tile_pool(name="outp", bufs=4))
        ps_s = ctx.enter_context(tc.tile_pool(name="ps_s", bufs=3, space="PSUM"))
        ps_q = ctx.enter_context(tc.tile_pool(name="ps_q", bufs=5, space="PSUM"))

        ident16 = const.tile([P, P], bf16)
        make_identity(nc, ident16)
        wtm_sb = const.tile([P, 1], f32)
        nc.sync.dma_start(wtm_sb, wtm)
        # replicated weight rows for DVE row-dot-products
        wt_rep = const.tile([P, H], f32)
        nc.gpsimd.dma_start(wt_rep, rep_rows(wt))
        wm_rep = const.tile([P, H], f32)
        nc.gpsimd.dma_start(wm_rep, rep_rows(wm))

        for b in range(BPC):
            # ---- masks, loaded directly as (P, chunks): [p, j] = mask[p*8+j]
            tmki = small.tile([P, LT], i32, tag="tmki")
            nc.sync.dma_start(tmki, tmask[b].rearrange("(p o) -> p o", p=P))
            bias2 = small.tile([P, LT], f32, tag="bias2")  # per-l
            tmkf = small.tile([P, LT], f32, tag="tmkf")
            nc.vector.tensor_copy(tmkf, tmki)
            nc.vector.tensor_scalar(bias2, tmkf, 1.0, NEGB,
                                    op0=Alu.subtract, op1=Alu.mult)
            mmki = small.tile([P, MT], i32, tag="mmki")
            nc.sync.dma_start(mmki, mmask[b].rearrange("(p o) -> p o", p=P))
            bias1 = small.tile([P, MT], f32, tag="bias1")  # per-m
            mmkf = small.tile([P, MT], f32, tag="mmkf")
            nc.vector.tensor_copy(mmkf, mmki)
            nc.vector.tensor_scalar(bias1, mmkf, 1.0, NEGB,
                                    op0=Alu.subtract, op1=Alu.mult)

            # ---- natural-layout loads (fp32, 4KB/partition) + bf16 casts ----
            txt = oper.tile([P, LT, H], f32, tag="txt")
            nc.sync.dma_start(txt, text[b].rearrange("(p o) h -> p o h", p=P))
            mods = oper.tile([P, MT, H], f32, tag="mods")
            nc.sync.dma_start(mods, modl[b].rearrange("(p o) h -> p o h", p=P))

            # bf16 [text | 1]: rhs of the q2 matmul
            txt16 = oper.tile([P, LT, H + 1], bf16, tag="txt16")
            nc.vector.memset(txt16[:, :, H : H + 1], 1.0)
            nc.vector.tensor_copy(txt16[:, :, :H], txt)
            # bf16 [mod | 1 | wq]: rhs of the fused a/b matmul (257 wide)
            modwq = big.tile([P, MT, 2 * H + 1], bf16, tag="modwq")
            nc.vector.memset(modwq[:, :, H : H + 1], 1.0)
            nc.vector.tensor_copy(modwq[:, :, :H], mods)

            # ---- s0/s1 row-dots on DVE (fp32), added into the bias columns ----
            s0col = small.tile([P, LT], f32, tag="s0col")
            for j in range(LT):
                scr = small.tile([P, H], f32, tag="scr")
                nc.vector.scalar_tensor_tensor(
                    out=scr, in0=txt[:, j, :], scalar=1.0, in1=wt_rep,
                    op0=Alu.mult, op1=Alu.mult,
                    accum_out=s0col[:, j : j + 1])
            nc.vector.tensor_add(bias2, bias2, s0col)
            s1col = small.tile([P, MT], f32, tag="s1col")
            for k in range(MT):
                scr = small.tile([P, H], f32, tag="scr")
                nc.vector.scalar_tensor_tensor(
                    out=scr, in0=mods[:, k, :], scalar=1.0, in1=wm_rep,
                    op0=Alu.mult, op1=Alu.mult,
                    accum_out=s1col[:, k : k + 1])
            nc.vector.tensor_add(bias1, bias1, s1col)

            # ---- transposes (bf16): txtT/modT are (H, L)/(H, M) ----
            # column c = j*128+p' of txtT holds row l = p'*8+j
            txtT = oper.tile([P, L], bf16, tag="txtT")
            modT = oper.tile([P, M], bf16, tag="modT")
            for j in range(LT):
                tp = ps_q.tile([P, P], bf16, tag="q")
                nc.tensor.transpose(tp, txt16[:, j, :H], ident16)
                nc.vector.tensor_copy(txtT[:, ts(j, P)], tp)
            for k in range(MT):
                tp = ps_q.tile([P, P], bf16, tag="q")
                nc.tensor.transpose(tp, modwq[:, k, :H], ident16)
                nc.vector.tensor_copy(modT[:, ts(k, P)], tp)

            # ---- X^T = txtT * w_tm (per-partition h scale), in place ----
            nc.vector.tensor_scalar_mul(txtT, txtT, wtm_sb)

            # ---- E2[l, m] = exp(s2 + bias2[l])  (L on partitions, bf16) ----
            E2 = big.tile([P, LT, M], bf16, tag="E2")
            for j in range(LT):
                for half in range(2):
                    hs = ts(half, 512)
                    sp = ps_s.tile([P, 512], f32, tag="s")
                    nc.tensor.matmul(sp, txtT[:, ts(j, P)], modT[:, hs],
                                     start=True, stop=True)
                    nc.scalar.activation(E2[:, j, hs], sp, Exp,
                                         bias=bias2[:, j : j + 1], scale=1.0)

            # ---- E1T[m, l] = exp(s2T + bias1[m]) interleaved with q2 ----
            # q2[m,:] = E2.T @ [text|1]; wq = q2/D2 (written into modwq)
            E1T = big.tile([P, MT, L], bf16, tag="E1T")
            for k in range(MT):
                for half in range(2):
                    hs = ts(half, 512)
                    sp = ps_s.tile([P, 512], f32, tag="s")
                    nc.tensor.matmul(sp, modT[:, ts(k, P)], txtT[:, hs],
                                     start=True, stop=True)
                    nc.scalar.activation(E1T[:, k, hs], sp, Exp,
                                         bias=bias1[:, k : k + 1], scale=1.0)
                qp = ps_q.tile([P, H + 1], f32, tag="q")
                for j in range(LT):
                    nc.tensor.matmul(qp, E2[:, j, ts(k, P)], txt16[:, j, :],
                                     start=(j == 0), stop=(j == LT - 1))
                rec = small.tile([P, 1], f32, tag="rec2")
                nc.vector.reciprocal(rec, qp[:, H : H + 1])
                nc.vector.tensor_scalar_mul(modwq[:, k, H + 1 :], qp[:, :H], rec)

            # ---- fused [a | D1 | b] = E1 @ [mod | 1 | wq]; assemble out ----
            for j in range(LT):
                pa = ps_q.tile([P, 2 * H + 1], f32, tag="q")
                for k in range(MT):
                    nc.tensor.matmul(pa, E1T[:, k, ts(j, P)], modwq[:, k, :],
                                     start=(k == 0), stop=(k == MT - 1))
                rec1 = small.tile([P, 1], f32, tag="rec1")
                nc.vector.reciprocal(rec1, pa[:, H : H + 1])
                o = outp.tile([P, 4 * H], f32, tag="o")
                nc.gpsimd.tensor_copy(o[:, 0:H], txt[:, j, :])
                nc.vector.tensor_scalar_mul(o[:, H : 2 * H], pa[:, :H], rec1)
                nc.vector.tensor_mul(o[:, 2 * H : 3 * H], o[:, H : 2 * H],
                                     txt[:, j, :])
                nc.vector.tensor_scalar_mul(o[:, 3 * H : 4 * H], pa[:, H + 1 :], rec1)
                nc.vector.tensor_mul(o[:, 3 * H : 4 * H], o[:, 3 * H : 4 * H],
                                     txt[:, j, :])
                nc.sync.dma_start(
                    out[b].rearrange("(p o) c -> p o c", p=P)[:, j, :], o
                )
    nc.compile()
    return nc


def get_nc():
    if "nc" not in _CACHE:
        _CACHE["nc"] = _build()
    return _CACHE["nc"]


def make_in_maps(text, modality, text_mask, modality_mask,
                 text_weight, modality_weight, text_modality_weight):
    text = np.ascontiguousarray(np.asarray(text, dtype=np.float32))
    modality = np.ascontiguousarray(np.asarray(modality, dtype=np.float32))
    text_mask = np.ascontiguousarray(np.asarray(text_mask, dtype=np.int32))
    modality_mask = np.ascontiguousarray(np.asarray(modality_mask, dtype=np.int32))
    wt = np.ascontiguousarray(np.asarray(text_weight, dtype=np.float32).reshape(H, 1))
    wm = np.ascontiguousarray(
        np.asarray(modality_weight, dtype=np.float32).reshape(H, 1))
    wtm = np.ascontiguousarray(
        np.asarray(text_modality_weight, dtype=np.float32).reshape(H, 1))
    in_maps = []
    for c in range(NCORES):
        sl = slice(BPC * c, BPC * (c + 1))
        in_maps.append({
            "text": np.ascontiguousarray(text[sl]),
            "modality": np.ascontiguousarray(modality[sl]),
            "text_mask": np.ascontiguousarray(text_mask[sl]),
            "modality_mask": np.ascontiguousarray(modality_mask[sl]),
            "w_text": wt,
            "w_mod": wm,
            "w_tm": wtm,
        })
    return in_maps


def kernel(text, modality, text_mask, modality_mask,
           text_weight, modality_weight, text_modality_weight, bias,
           trace=False):
    from concourse.bass_utils import run_bass_kernel_spmd

    nc = get_nc()
    in_maps = make_in_maps(text, modality, text_mask, modality_mask,
                           text_weight, modality_weight, text_modality_weight)
    res = run_bass_kernel_spmd(nc, in_maps, core_ids=list(range(NCORES)),
                               trace=trace)
    outp = np.concatenate([r["out"] for r in res.results], axis=0)
    if trace:
        kernel.last_result = res
    return outp
